# revision 4
# baseline (speedup 1.0000x reference)
"""Trainium2 Bass kernel for the spatial-attention module.

Reference computation (B=32, HS=512, C=256, H=W=64, A=256):
    wh     = h_dec @ W_h + b_h                      # (B, A)
    wfm    = einsum('bchw,ca->bhwa', fm, W_fm) + b_fm
    scores = einsum('bhwa,ba->bhw', wfm, wh)
    normed = softmax(scores over h*w)
    out    = einsum('bchw,bhw->bc', fm, normed)     # (B, C)

Refactor: scores = einsum('bchw,bc->bhw', fm, v) with
v = einsum('ca,ba->bc', W_fm, wh); the per-sample constant (b_fm . wh)
cancels inside softmax, so b_fm is never needed.  One HBM read of fm,
resident in SBUF afterwards.

Precision (rel-err budget 2e-2, achieves ~5e-3):
  - fm converted to fp16 on the host: halves HBM traffic and runs the
    PE moving operand at full (bf16-equal, HW-measured) rate.  fp16's
    11-bit mantissa keeps softmax scores accurate; bf16 does not.
  - exp() output e is bf16 (spans ~e^24: overflows fp16, not bf16).
  - All accumulations (PSUM scores, softmax Z, context partials) fp32.

Engine split for the context pass (HW-microbenched):
  scalar_tensor_tensor (fused mult+accum) is 1x-only on DVE, while
  plain tensor_tensor hits the 2x 16-bit mode, so DVE time is the
  bottleneck.  Per sample, 3 of the 4 (group, cc) units run as fused
  STT on DVE; the 4th unit's multiply runs on the otherwise-idle
  GpSimd engine and its pixel-sum on the Scalar engine (ACT Copy with
  accum_out), keeping every engine under the ~7 us/sample HBM stream
  cadence.

Sharding: data-parallel over batch, 4 samples per core, no comms.
"""

import numpy as np

import concourse.bacc as bacc
import concourse.bass as bass
import concourse.tile as tile
from concourse import bass_utils, mybir
from concourse.masks import make_identity

F32 = mybir.dt.float32
F32R = mybir.dt.float32r
F16 = mybir.dt.float16
BF16 = mybir.dt.bfloat16

N_CORES = 8
B = 32
BS = B // N_CORES
HS = 512
C = 256
A = 256
NPIX = 64 * 64
CP = 128
CC = C // CP
AC = A // CP
KC = HS // CP
PCH = 512  # pixels per matmul chunk (one PSUM bank)
GRP = 2048  # pixels per exp/context group (4 PSUM banks)
SOFTMAX_SHIFT = 60.0


def _piece_layout(b):
    """fm DMA piece spans (pixel_offset, npix) for sample b, per cc chunk.

    2D access patterns (one piece per (b, cc)) keep the Sync engine's
    descriptor generation fast.  The first sample is split for an early
    compute start; the last tapers so little dependent compute remains
    after the HBM stream ends.
    """
    if b == 0:
        return [(0, 2048), (2048, 2048)]
    if b == BS - 1:
        return [(0, 2048), (2048, 1024), (3072, 512), (3584, 512)]
    return [(0, 4096)]


def _group_layout(b):
    """exp/context groups (pixel_offset, npix, piece_idx), inside one piece."""
    groups = []
    for pi, (off, npx) in enumerate(_piece_layout(b)):
        o = 0
        while o < npx:
            n = min(GRP, npx - o)
            groups.append((off + o, n, pi))
            o += n
    return groups


NGMAX = 4


def _build_program():
    nc = bacc.Bacc("TRN2", target_bir_lowering=False, debug=False)

    h_dec_d = nc.dram_tensor("h_dec", (BS, HS), F32, kind="ExternalInput")
    fm_d = nc.dram_tensor("fm", (BS, C, 64, 64), F16, kind="ExternalInput")
    w_fm_d = nc.dram_tensor("W_fm", (C, A), F32, kind="ExternalInput")
    w_h_d = nc.dram_tensor("W_h", (HS, A), F32R, kind="ExternalInput")
    b_h_d = nc.dram_tensor("b_h", (A,), F32R, kind="ExternalInput")
    out_d = nc.dram_tensor("out", (BS, C), F32, kind="ExternalOutput")

    with tile.TileContext(nc) as tc:
        with (
            tc.tile_pool(name="consts", bufs=1) as consts,
            tc.tile_pool(name="wpool", bufs=1) as wpool,
            tc.tile_pool(name="fmpool", bufs=1) as fmpool,
            tc.tile_pool(name="smax", bufs=4) as smax,
            tc.tile_pool(name="scratch", bufs=2) as scratch_pool,
            tc.tile_pool(name="psum", bufs=1, space="PSUM") as pp,
        ):
            # ---- weight DMAs first (0.77 MB lead-in), then the fm stream
            h_dec_sb = wpool.tile([BS, HS], F32)
            nc.sync.dma_start(out=h_dec_sb, in_=h_dec_d.ap())
            w_h_sb = wpool.tile([128, KC, A], F32R)
            nc.sync.dma_start(
                out=w_h_sb, in_=w_h_d.ap().rearrange("(kc kp) a -> kp kc a", kp=128)
            )
            b_h_sb = wpool.tile([1, A], F32R)
            nc.sync.dma_start(out=b_h_sb, in_=b_h_d.ap().rearrange("(o a) -> o a", o=1))
            w_fm_sb = wpool.tile([128, CC, A], F32)
            nc.sync.dma_start(
                out=w_fm_sb, in_=w_fm_d.ap().rearrange("(cc cp) a -> cp cc a", cp=128)
            )

            # ---- fm resident in SBUF, fp16, one 2D piece per (b, cc, span)
            fm_v = fm_d.ap().rearrange("b (cc cp) h w -> b cc cp (h w)", cp=128)
            fm_sb = {}
            for b in range(BS):
                for pi, (off, npx) in enumerate(_piece_layout(b)):
                    for cc in range(CC):
                        t = fmpool.tile(
                            [128, npx], F16,
                            name=f"fm_{b}_{cc}_{pi}", tag=f"fm_{b}_{cc}_{pi}",
                        )
                        nc.sync.dma_start(out=t, in_=fm_v[b, cc, :, off : off + npx])
                        fm_sb[(b, cc, pi)] = t

            # ---- constants ------------------------------------------------
            identity = consts.tile([128, 128], F32)
            make_identity(nc, identity)
            ones4_f = consts.tile([1, BS], F32)
            nc.vector.memset(ones4_f, 1.0)
            ones4 = consts.tile([1, BS], F32R)
            nc.scalar.copy(ones4, ones4_f)
            negshift = consts.tile([128, 1], F32)
            nc.vector.memset(negshift, -SOFTMAX_SHIFT)
            one_col = consts.tile([128, 1], F32)
            nc.vector.memset(one_col, 1.0)

            # ---- phase 0: whT[a,b] = (h_dec @ W_h + b_h).T ----------------
            # PSUM comes from the same 4-bank "scores" tag the main loop
            # uses (phase 0/1 is over before the first scores matmul).
            def mm_psum():
                t = pp.tile([128, GRP], F32, tag="scores", bufs=2)
                return t

            hdT_full = mm_psum()
            for kc in range(KC):
                nc.tensor.transpose(
                    hdT_full[:, kc * BS : (kc + 1) * BS],
                    h_dec_sb[:, kc * 128 : (kc + 1) * 128],
                    identity[0:BS, 0:BS],
                )
            hdT_sb = wpool.tile([128, KC, BS], F32R)
            nc.scalar.copy(
                hdT_sb, hdT_full[:, 0 : KC * BS].rearrange("p (kc b) -> p kc b", kc=KC)
            )

            whT_sb = wpool.tile([128, AC, BS], F32R)
            for ac in range(AC):
                whT_ps = mm_psum()
                for kc in range(KC):
                    nc.tensor.matmul(
                        whT_ps[:, 0:BS],
                        w_h_sb[:, kc, ac * 128 : (ac + 1) * 128],
                        hdT_sb[:, kc, :],
                        start=(kc == 0),
                        stop=False,
                    )
                nc.tensor.matmul(
                    whT_ps[:, 0:BS],
                    b_h_sb[0:1, ac * 128 : (ac + 1) * 128],
                    ones4,
                    start=False,
                    stop=True,
                )
                nc.scalar.copy(whT_sb[:, ac, :], whT_ps[:, 0:BS])

            # ---- phase 1: vT[c,b] = sum_a W_fm[c,a] * wh[b,a], cast fp16 --
            wfmT_sb = wpool.tile([128, AC, CC, 128], F32R)
            for cc in range(CC):
                wfmT_ps = mm_psum()
                for ac in range(AC):
                    nc.tensor.transpose(
                        wfmT_ps[:, ac * 128 : (ac + 1) * 128],
                        w_fm_sb[:, cc, ac * 128 : (ac + 1) * 128],
                        identity,
                    )
                for ac in range(AC):
                    nc.scalar.copy(
                        wfmT_sb[:, ac, cc, :], wfmT_ps[:, ac * 128 : (ac + 1) * 128]
                    )

            vT_sb = wpool.tile([128, CC, BS], F16)
            for cc in range(CC):
                vT_ps = mm_psum()
                for ac in range(AC):
                    nc.tensor.matmul(
                        vT_ps[:, 0:BS],
                        wfmT_sb[:, ac, cc, :],
                        whT_sb[:, ac, :],
                        start=(ac == 0),
                        stop=(ac == AC - 1),
                    )
                nc.scalar.copy(vT_sb[:, cc, :], vT_ps[:, 0:BS])

            # ---- main per-sample pipeline ---------------------------------
            # scores come out of PE replicated on all 128 partitions (vT
            # broadcast stationary), so the exp output is directly the
            # broadcast operand the context multiply needs.  softmax
            # shift-invariance lets a compile-time -SOFTMAX_SHIFT bias
            # replace the data max.
            ctx_sb = wpool.tile([128, BS, CC], F32)
            out_v = out_d.ap().rearrange("b (cc cp) -> cp b cc", cp=128)
            for b in range(BS):
                groups = _group_layout(b)
                ng = len(groups)
                zparts = smax.tile([128, NGMAX], F32, tag="zparts", bufs=2)
                parts = smax.tile([128, CC, NGMAX], F32, tag="parts", bufs=2)
                e_tiles = []
                # one (group, cc) unit per sample is offloaded:
                # multiply on GpSimd, pixel-sum on Scalar
                off_g = 0
                for g, (goff, gnpx, pi) in enumerate(groups):
                    lo = goff - _piece_layout(b)[pi][0]
                    sc_ps = pp.tile([128, GRP], F32, tag="scores", bufs=2)
                    for h in range((gnpx + PCH - 1) // PCH):
                        co = h * PCH
                        cn = min(PCH, gnpx - co)
                        for cc in range(CC):
                            nc.tensor.matmul(
                                sc_ps[:, co : co + cn],
                                vT_sb[:, cc, b : b + 1].to_broadcast((128, 128)),
                                fm_sb[(b, cc, pi)][:, lo + co : lo + co + cn],
                                start=(cc == 0),
                                stop=(cc == CC - 1),
                            )
                    e_big = smax.tile([128, GRP], BF16, tag="e", bufs=3)
                    nc.scalar.activation(
                        e_big[:, :gnpx], sc_ps[:, :gnpx],
                        mybir.ActivationFunctionType.Exp,
                        bias=negshift, scale=1.0,
                        accum_out=zparts[:, g : g + 1],
                    )
                    e_tiles.append(e_big)
                    for cc in range(CC):
                        src = fm_sb[(b, cc, pi)][:, lo : lo + gnpx]
                        if g == off_g and cc == 1:
                            prod = scratch_pool.tile(
                                [128, GRP], BF16, tag="prod", bufs=2
                            )
                            nc.gpsimd.tensor_tensor(
                                out=prod[:, :gnpx], in0=src, in1=e_big[:, :gnpx],
                                op=mybir.AluOpType.mult,
                            )
                        else:
                            scr = scratch_pool.tile([128, GRP], F16, tag="scr", bufs=3)
                            nc.vector.scalar_tensor_tensor(
                                out=scr[:, :gnpx],
                                in0=src,
                                scalar=one_col,
                                in1=e_big[:, :gnpx],
                                op0=mybir.AluOpType.mult,
                                op1=mybir.AluOpType.mult,
                                accum_out=parts[:, cc, g : g + 1],
                            )

                # offloaded unit's pixel-sum on the Scalar engine (after
                # the sample's exps so it does not block them in-queue)
                goff, gnpx, pi = groups[off_g]
                prod_s = scratch_pool.tile(
                    [128, GRP], BF16, name=f"prod_s_{b}", tag="prod_s", bufs=2
                )
                nc.scalar.activation(
                    prod_s[:, :gnpx],
                    prod[:, :gnpx],
                    mybir.ActivationFunctionType.Copy,
                    accum_out=parts[:, 1, off_g : off_g + 1],
                )

                # Z (replicated on all partitions) and final scale by 1/Z
                z_rep = smax.tile([128, 1], F32, tag="z")
                nc.vector.tensor_reduce(
                    z_rep, zparts[:, :ng], axis=mybir.AxisListType.X,
                    op=mybir.AluOpType.add,
                )
                rz_rep = smax.tile([128, 1], F32, tag="rz")
                nc.vector.reciprocal(rz_rep, z_rep)
                for cc in range(CC):
                    pr = smax.tile([128, 1], F32, tag="pr")
                    nc.vector.tensor_reduce(
                        pr, parts[:, cc, :ng], axis=mybir.AxisListType.X,
                        op=mybir.AluOpType.add,
                    )
                    nc.scalar.mul(ctx_sb[:, b, cc : cc + 1], pr, rz_rep)
                # per-sample output DMA: samples 0..BS-2 flush during the
                # stream; only the last sample's 1KB write is in the tail
                nc.sync.dma_start(
                    out=out_v[:, b : b + 1, :], in_=ctx_sb[:, b : b + 1, :]
                )

    nc.compile()
    return nc


_NC_CACHE = None


def _get_program():
    global _NC_CACHE
    if _NC_CACHE is None:
        _NC_CACHE = _build_program()
    return _NC_CACHE


def kernel(**inputs):
    h_dec = np.ascontiguousarray(np.asarray(inputs["h_dec"], dtype=np.float32))
    fm16 = np.asarray(inputs["fm"], dtype=np.float32).astype(np.float16)
    w_fm = np.ascontiguousarray(np.asarray(inputs["W_fm"], dtype=np.float32))
    w_h = np.ascontiguousarray(np.asarray(inputs["W_h"], dtype=np.float32))
    b_h = np.ascontiguousarray(np.asarray(inputs["b_h"], dtype=np.float32))

    nc = _get_program()
    in_maps = []
    for c in range(N_CORES):
        sl = slice(c * BS, (c + 1) * BS)
        in_maps.append(
            {
                "h_dec": np.ascontiguousarray(h_dec[sl]),
                "fm": np.ascontiguousarray(fm16[sl]),
                "W_fm": w_fm,
                "W_h": w_h,
                "b_h": b_h,
            }
        )
    res = bass_utils.run_bass_kernel_spmd(nc, in_maps, core_ids=list(range(N_CORES)))
    return np.concatenate([r["out"] for r in res.results], axis=0)


# revision 5
# speedup vs baseline: 1.2990x; 1.2990x over previous
"""Trainium2 Bass kernel for the spatial-attention module.

Reference computation (B=32, HS=512, C=256, H=W=64, A=256):
    wh     = h_dec @ W_h + b_h                      # (B, A)
    wfm    = einsum('bchw,ca->bhwa', fm, W_fm) + b_fm
    scores = einsum('bhwa,ba->bhw', wfm, wh)
    normed = softmax(scores over h*w)
    out    = einsum('bchw,bhw->bc', fm, normed)     # (B, C)

Refactor: scores = einsum('bchw,bc->bhw', fm, v) with
v = einsum('ca,ba->bc', W_fm, wh); the per-sample constant (b_fm . wh)
cancels inside softmax, so b_fm is never needed.  One HBM read of fm,
resident in SBUF afterwards.

Precision (rel-err budget 2e-2, achieves ~5e-3):
  - fm converted to fp16 on the host: halves HBM traffic and runs the
    PE moving operand at full (bf16-equal, HW-measured) rate.  fp16's
    11-bit mantissa keeps softmax scores accurate; bf16 does not.
  - exp() output e is bf16 (spans ~e^24: overflows fp16, not bf16).
  - All accumulations (PSUM scores, softmax Z, context partials) fp32.

Engine split for the context pass (HW-microbenched):
  scalar_tensor_tensor (fused mult+accum) is 1x-only on DVE, while
  plain tensor_tensor hits the 2x 16-bit mode, so DVE time is the
  bottleneck.  Per sample, 3 of the 4 (group, cc) units run as fused
  STT on DVE; the 4th unit's multiply runs on the otherwise-idle
  GpSimd engine and its pixel-sum on the Scalar engine (ACT Copy with
  accum_out), keeping every engine under the ~7 us/sample HBM stream
  cadence.

Sharding: data-parallel over batch, 4 samples per core, no comms.
"""

import numpy as np

import concourse.bacc as bacc
import concourse.bass as bass
import concourse.tile as tile
from concourse import bass_utils, mybir
from concourse.masks import make_identity

F32 = mybir.dt.float32
F32R = mybir.dt.float32r
F16 = mybir.dt.float16
BF16 = mybir.dt.bfloat16

N_CORES = 8
B = 32
BS = B // N_CORES
HS = 512
C = 256
A = 256
NPIX = 64 * 64
CP = 128
CC = C // CP
AC = A // CP
KC = HS // CP
PCH = 512  # pixels per matmul chunk (one PSUM bank)
GRP = 2048  # pixels per exp/context group (4 PSUM banks)
SOFTMAX_SHIFT = 60.0


def _piece_layout(b):
    """fm DMA piece spans (pixel_offset, npix) for sample b, per cc chunk.

    2D access patterns (one piece per (b, cc)) keep the Sync engine's
    descriptor generation fast.  The first sample is split for an early
    compute start; the last tapers so little dependent compute remains
    after the HBM stream ends.
    """
    if b == 0:
        return [(0, 2048), (2048, 2048)]
    if b == BS - 1:
        return [(0, 2048), (2048, 1024), (3072, 512), (3584, 512)]
    return [(0, 4096)]


def _group_layout(b):
    """exp/context groups (pixel_offset, npix, piece_idx), inside one piece."""
    groups = []
    for pi, (off, npx) in enumerate(_piece_layout(b)):
        o = 0
        while o < npx:
            n = min(GRP, npx - o)
            groups.append((off + o, n, pi))
            o += n
    return groups


NGMAX = 4


def _build_program():
    nc = bacc.Bacc("TRN2", target_bir_lowering=False, debug=False)

    h_dec_d = nc.dram_tensor("h_dec", (BS, HS), F32, kind="ExternalInput")
    fm_d = nc.dram_tensor("fm", (BS, C, 64, 64), F16, kind="ExternalInput")
    w_fm_d = nc.dram_tensor("W_fm", (C, A), F32, kind="ExternalInput")
    w_h_d = nc.dram_tensor("W_h", (HS, A), F32R, kind="ExternalInput")
    b_h_d = nc.dram_tensor("b_h", (A,), F32R, kind="ExternalInput")
    out_d = nc.dram_tensor("out", (BS, C), F32, kind="ExternalOutput")

    with tile.TileContext(nc) as tc:
        with (
            tc.tile_pool(name="consts", bufs=1) as consts,
            tc.tile_pool(name="wpool", bufs=1) as wpool,
            tc.tile_pool(name="fmpool", bufs=1) as fmpool,
            tc.tile_pool(name="smax", bufs=4) as smax,
            tc.tile_pool(name="scratch", bufs=2) as scratch_pool,
            tc.tile_pool(name="psum", bufs=1, space="PSUM") as pp,
        ):
            # ---- weight DMAs first (0.77 MB lead-in), then the fm stream
            h_dec_sb = wpool.tile([BS, HS], F32)
            nc.sync.dma_start(out=h_dec_sb, in_=h_dec_d.ap())
            w_h_sb = wpool.tile([128, KC, A], F32R)
            nc.sync.dma_start(
                out=w_h_sb, in_=w_h_d.ap().rearrange("(kc kp) a -> kp kc a", kp=128)
            )
            b_h_sb = wpool.tile([1, A], F32R)
            nc.sync.dma_start(out=b_h_sb, in_=b_h_d.ap().rearrange("(o a) -> o a", o=1))
            w_fm_sb = wpool.tile([128, CC, A], F32)
            nc.sync.dma_start(
                out=w_fm_sb, in_=w_fm_d.ap().rearrange("(cc cp) a -> cp cc a", cp=128)
            )

            # ---- fm resident in SBUF, fp16, one 2D piece per (b, cc, span)
            fm_v = fm_d.ap().rearrange("b (cc cp) h w -> b cc cp (h w)", cp=128)
            fm_sb = {}
            for b in range(BS):
                for pi, (off, npx) in enumerate(_piece_layout(b)):
                    for cc in range(CC):
                        t = fmpool.tile(
                            [128, npx], F16,
                            name=f"fm_{b}_{cc}_{pi}", tag=f"fm_{b}_{cc}_{pi}",
                        )
                        nc.sync.dma_start(out=t, in_=fm_v[b, cc, :, off : off + npx])
                        fm_sb[(b, cc, pi)] = t

            # ---- constants ------------------------------------------------
            identity = consts.tile([128, 128], F32)
            make_identity(nc, identity)
            ones4_f = consts.tile([1, BS], F32)
            nc.vector.memset(ones4_f, 1.0)
            ones4 = consts.tile([1, BS], F32R)
            nc.scalar.copy(ones4, ones4_f)
            negshift = consts.tile([128, 1], F32)
            nc.vector.memset(negshift, -SOFTMAX_SHIFT)
            one_col = consts.tile([128, 1], F32)
            nc.vector.memset(one_col, 1.0)

            # ---- phase 0: whT[a,b] = (h_dec @ W_h + b_h).T ----------------
            # PSUM comes from the same 4-bank "scores" tag the main loop
            # uses (phase 0/1 is over before the first scores matmul).
            def mm_psum():
                t = pp.tile([128, GRP], F32, tag="scores", bufs=2)
                return t

            hdT_full = mm_psum()
            for kc in range(KC):
                nc.tensor.transpose(
                    hdT_full[:, kc * BS : (kc + 1) * BS],
                    h_dec_sb[:, kc * 128 : (kc + 1) * 128],
                    identity[0:BS, 0:BS],
                )
            hdT_sb = wpool.tile([128, KC, BS], F32R)
            nc.scalar.copy(
                hdT_sb, hdT_full[:, 0 : KC * BS].rearrange("p (kc b) -> p kc b", kc=KC)
            )

            whT_sb = wpool.tile([128, AC, BS], F32R)
            for ac in range(AC):
                whT_ps = mm_psum()
                for kc in range(KC):
                    nc.tensor.matmul(
                        whT_ps[:, 0:BS],
                        w_h_sb[:, kc, ac * 128 : (ac + 1) * 128],
                        hdT_sb[:, kc, :],
                        start=(kc == 0),
                        stop=False,
                    )
                nc.tensor.matmul(
                    whT_ps[:, 0:BS],
                    b_h_sb[0:1, ac * 128 : (ac + 1) * 128],
                    ones4,
                    start=False,
                    stop=True,
                )
                nc.scalar.copy(whT_sb[:, ac, :], whT_ps[:, 0:BS])

            # ---- phase 1: vT[c,b] = sum_a W_fm[c,a] * wh[b,a], cast fp16 --
            wfmT_sb = wpool.tile([128, AC, CC, 128], F32R)
            for cc in range(CC):
                wfmT_ps = mm_psum()
                for ac in range(AC):
                    nc.tensor.transpose(
                        wfmT_ps[:, ac * 128 : (ac + 1) * 128],
                        w_fm_sb[:, cc, ac * 128 : (ac + 1) * 128],
                        identity,
                    )
                for ac in range(AC):
                    nc.scalar.copy(
                        wfmT_sb[:, ac, cc, :], wfmT_ps[:, ac * 128 : (ac + 1) * 128]
                    )

            vT_sb = wpool.tile([128, CC, BS], F16)
            for cc in range(CC):
                vT_ps = mm_psum()
                for ac in range(AC):
                    nc.tensor.matmul(
                        vT_ps[:, 0:BS],
                        wfmT_sb[:, ac, cc, :],
                        whT_sb[:, ac, :],
                        start=(ac == 0),
                        stop=(ac == AC - 1),
                    )
                nc.scalar.copy(vT_sb[:, cc, :], vT_ps[:, 0:BS])

            # ---- main per-sample pipeline ---------------------------------
            # scores come out of PE replicated on all 128 partitions (vT
            # broadcast stationary), so the exp output is directly the
            # broadcast operand the context multiply needs.  softmax
            # shift-invariance lets a compile-time -SOFTMAX_SHIFT bias
            # replace the data max.
            ctx_sb = wpool.tile([128, BS, CC], F32)
            out_v = out_d.ap().rearrange("b (cc cp) -> cp b cc", cp=128)
            for b in range(BS):
                groups = _group_layout(b)
                ng = len(groups)
                zparts = smax.tile([128, NGMAX], F32, tag="zparts", bufs=2)
                parts = smax.tile([128, CC, NGMAX], F32, tag="parts", bufs=2)
                e_tiles = []
                # one (group, cc) unit per sample is offloaded:
                # multiply on GpSimd, pixel-sum on Scalar
                off_g = 0
                for g, (goff, gnpx, pi) in enumerate(groups):
                    lo = goff - _piece_layout(b)[pi][0]
                    sc_ps = pp.tile([128, GRP], F32, tag="scores", bufs=2)
                    for h in range((gnpx + PCH - 1) // PCH):
                        co = h * PCH
                        cn = min(PCH, gnpx - co)
                        for cc in range(CC):
                            nc.tensor.matmul(
                                sc_ps[:, co : co + cn],
                                vT_sb[:, cc, b : b + 1].to_broadcast((128, 128)),
                                fm_sb[(b, cc, pi)][:, lo + co : lo + co + cn],
                                start=(cc == 0),
                                stop=(cc == CC - 1),
                            )
                    e_big = smax.tile([128, GRP], BF16, tag="e", bufs=3)
                    nc.scalar.activation(
                        e_big[:, :gnpx], sc_ps[:, :gnpx],
                        mybir.ActivationFunctionType.Exp,
                        bias=negshift, scale=1.0,
                        accum_out=zparts[:, g : g + 1],
                    )
                    e_tiles.append(e_big)
                    for cc in range(CC):
                        src = fm_sb[(b, cc, pi)][:, lo : lo + gnpx]
                        if g == off_g and cc == 1:
                            # multiply at DVE 2x (plain TT), pixel-sum on
                            # the Scalar engine (GpSimd is unusable here:
                            # it shares an exclusive SBUF port pair with
                            # DVE 2-tensor ops)
                            prod = scratch_pool.tile(
                                [128, GRP], BF16, tag="prod", bufs=2
                            )
                            nc.vector.tensor_tensor(
                                out=prod[:, :gnpx], in0=src, in1=e_big[:, :gnpx],
                                op=mybir.AluOpType.mult,
                            )
                        else:
                            scr = scratch_pool.tile([128, GRP], F16, tag="scr", bufs=3)
                            nc.vector.scalar_tensor_tensor(
                                out=scr[:, :gnpx],
                                in0=src,
                                scalar=one_col,
                                in1=e_big[:, :gnpx],
                                op0=mybir.AluOpType.mult,
                                op1=mybir.AluOpType.mult,
                                accum_out=parts[:, cc, g : g + 1],
                            )

                # offloaded unit's pixel-sum on the Scalar engine (after
                # the sample's exps so it does not block them in-queue)
                goff, gnpx, pi = groups[off_g]
                prod_s = scratch_pool.tile(
                    [128, GRP], BF16, name=f"prod_s_{b}", tag="prod_s", bufs=2
                )
                nc.scalar.activation(
                    prod_s[:, :gnpx],
                    prod[:, :gnpx],
                    mybir.ActivationFunctionType.Copy,
                    accum_out=parts[:, 1, off_g : off_g + 1],
                )

                # Z (replicated on all partitions) and final scale by 1/Z
                z_rep = smax.tile([128, 1], F32, tag="z")
                nc.vector.tensor_reduce(
                    z_rep, zparts[:, :ng], axis=mybir.AxisListType.X,
                    op=mybir.AluOpType.add,
                )
                rz_rep = smax.tile([128, 1], F32, tag="rz")
                nc.vector.reciprocal(rz_rep, z_rep)
                for cc in range(CC):
                    pr = smax.tile([128, 1], F32, tag="pr")
                    nc.vector.tensor_reduce(
                        pr, parts[:, cc, :ng], axis=mybir.AxisListType.X,
                        op=mybir.AluOpType.add,
                    )
                    nc.scalar.mul(ctx_sb[:, b, cc : cc + 1], pr, rz_rep)
                # per-sample output DMA: samples 0..BS-2 flush during the
                # stream; only the last sample's 1KB write is in the tail
                nc.sync.dma_start(
                    out=out_v[:, b : b + 1, :], in_=ctx_sb[:, b : b + 1, :]
                )

    nc.compile()
    return nc


_NC_CACHE = None


def _get_program():
    global _NC_CACHE
    if _NC_CACHE is None:
        _NC_CACHE = _build_program()
    return _NC_CACHE


def kernel(**inputs):
    h_dec = np.ascontiguousarray(np.asarray(inputs["h_dec"], dtype=np.float32))
    fm16 = np.asarray(inputs["fm"], dtype=np.float32).astype(np.float16)
    w_fm = np.ascontiguousarray(np.asarray(inputs["W_fm"], dtype=np.float32))
    w_h = np.ascontiguousarray(np.asarray(inputs["W_h"], dtype=np.float32))
    b_h = np.ascontiguousarray(np.asarray(inputs["b_h"], dtype=np.float32))

    nc = _get_program()
    in_maps = []
    for c in range(N_CORES):
        sl = slice(c * BS, (c + 1) * BS)
        in_maps.append(
            {
                "h_dec": np.ascontiguousarray(h_dec[sl]),
                "fm": np.ascontiguousarray(fm16[sl]),
                "W_fm": w_fm,
                "W_h": w_h,
                "b_h": b_h,
            }
        )
    res = bass_utils.run_bass_kernel_spmd(nc, in_maps, core_ids=list(range(N_CORES)))
    return np.concatenate([r["out"] for r in res.results], axis=0)
